# revision 16
# baseline (speedup 1.0000x reference)
"""Trainium2 Bass kernel for gated multi-head attention with pair bias.

Reference computation (B=2, S=2048, C_IN=512, H=8, C=64):
    q,k,v = heads(x @ Wq), heads(x @ Wk), heads(x @ Wv)
    logits = q k^T / sqrt(C) + bias + mask_offset
    attn   = softmax(logits)
    o      = attn @ v
    out    = (sigmoid(x @ Wg + bg) * concat(o)) @ Wo + bo

Sharding: 8 cores = 2 batches x 4 head-pairs. Core c handles batch c//4,
heads (2*(c%4), 2*(c%4)+1). Weights are sliced per-core on the host; each
core computes a partial output (sum over its two heads) and the host sums
4 partials per batch and adds bo.

Device math notes:
  - Everything is computed in the "transposed" orientation (feature dim on
    partitions) so the PE contraction dims line up without any on-device
    transposes: qT/kT/gT = W^T x^T via matmul(lhsT=W_chunk, rhs=xT_chunk).
  - softmax skips the max-subtraction (logits are O(+-8): exp is safe in
    fp32) and uses exp(qk) * exp(bias), with exp(bias^T) precomputed on the
    host in bf16. The key mask is folded into v (v*m) and the softmax
    denominator comes from an extra ones*m column appended to v, so the
    attention matmul produces [o_unnorm ; rowsum] in one accumulation.
  - All matmuls run in bf16 (fp32 PSUM accumulation).
"""

import os
import sys
import threading

import numpy as np

sys.path.insert(0, "/opt/trn_rl_repo")

import ml_dtypes

import concourse.bass as bass
import concourse.tile as tile
from concourse import mybir
from concourse.bass_utils import run_bass_kernel_spmd

# ---------------------------------------------------------------------------
# This toolchain's walrus encodes at most ONE semaphore wait per Drain/CTRL
# instruction; Tile's end-of-kernel drain can carry several (one per DMA
# queue). Split them across a chain of single-wait drains.
# ---------------------------------------------------------------------------


_NOP_UID = [0]


def _split_multi_waits(nc):
    """Rewrite every instruction carrying >1 sem waits: keep one wait on the
    instruction, hoist the others onto same-engine NoOps inserted right
    before it (engine streams execute in order, so this is equivalent)."""
    for fn in nc.m.functions:
        for bb in fn.blocks:
            insts = list(bb.instructions)
            out = []
            changed = False
            for inst in insts:
                si = inst.sync_info
                if si is not None and len(si.on_wait) > 1:
                    changed = True
                    waits = list(si.on_wait)
                    si.on_wait = waits[:1]
                    for w in waits[1:]:
                        _NOP_UID[0] += 1
                        nop = mybir.InstNoOp(
                            name=f"waitsplit-{_NOP_UID[0]}",
                            engine=inst.engine,
                            ins=[],
                            outs=[],
                        )
                        nop.sync_info = mybir.SyncInfo(on_wait=[w], on_update=[])
                        out.append(nop)
                out.append(inst)
            if changed:
                bb.instructions = out


def _drain_and_barrier_split(self, tick_clock, wait_clock):
    from concourse.vector_clock import ScopedClock

    drain_inst = self.nc.sync.drain()
    wait_clock.add_sem_waits(
        drain_inst.ins, ScopedClock({None: tick_clock.global_clock})
    )
    si = drain_inst.ins.sync_info
    if si is not None and len(si.on_wait) > 1:
        extra = list(si.on_wait[1:])
        si.on_wait = list(si.on_wait[:1])
        for w in extra:
            d2 = self.nc.sync.drain()
            d2.ins.sync_info = mybir.SyncInfo(on_wait=[w], on_update=[])

    self.nc.all_engine_barrier()
    assert self.sems is not None
    popped = self.nc._tile_sem_poison_stack.pop()
    assert popped is self._sem_poison
    self.nc.clear_and_free_semaphores(list(self.sems.allocated().values()))
    self.nc.all_engine_barrier()

    _split_multi_waits(self.nc)


tile.TileContext._drain_and_barrier = _drain_and_barrier_split

BF16 = mybir.dt.bfloat16
F32 = mybir.dt.float32
NBF = ml_dtypes.bfloat16

B, S, C_IN, H, C = 2, 2048, 512, 8, 64
P = 128
NKT = S // P  # 16 key tiles
NCI = C_IN // P  # 4 contraction chunks
QH = 1024  # q-chunk processed per exp instruction
NQH = S // QH  # 2

Exp = mybir.ActivationFunctionType.Exp
Tanh = mybir.ActivationFunctionType.Tanh


def _build_nc():
    nc = bass.Bass("TRN2")

    x_t = nc.dram_tensor("xt", [NCI, P, S], BF16, kind="ExternalInput")
    w_q = nc.dram_tensor("wq", [NCI, P, P], BF16, kind="ExternalInput")
    w_k = nc.dram_tensor("wk", [NCI, P, P], BF16, kind="ExternalInput")
    w_g = nc.dram_tensor("wg", [NCI, P, P], BF16, kind="ExternalInput")
    w_v = nc.dram_tensor("wv", [NCI, P, P], BF16, kind="ExternalInput")
    bg_t = nc.dram_tensor("bgv", [P, 1], F32, kind="ExternalInput")
    mc_t = nc.dram_tensor("maskcol", [P, NKT], F32, kind="ExternalInput")
    eb_t = nc.dram_tensor("ebias", [2, S, S], BF16, kind="ExternalInput")
    wo_t = nc.dram_tensor("wo", [P, C_IN], BF16, kind="ExternalInput")
    out_t = nc.dram_tensor("out", [S, C_IN], F32, kind="ExternalOutput")

    from contextlib import ExitStack

    with tile.TileContext(nc) as tc, ExitStack() as ctx:
        const = ctx.enter_context(tc.tile_pool(name="const", bufs=1))
        drp = ctx.enter_context(tc.tile_pool(name="dram", bufs=2, space="DRAM"))

        # ---------------- persistent SBUF tiles ----------------
        # weights first (tiny), then x^T in column halves so the first
        # projection chunks can start before the whole input has landed
        def load_w(dram):
            t = const.tile([P, NCI, P], BF16, tag=f"w{dram.name}", name=f"wsb_{dram.name}")
            nc.sync.dma_start(t[:], dram[:].rearrange("c p m -> p c m"))
            return t

        wq_sb = load_w(w_q)
        xt_sb = [const.tile([P, S], BF16, tag=f"xt{i}", name=f"xtsb{i}") for i in range(NCI)]
        hs0 = slice(0, S // 2)
        for i in range(NCI):
            nc.sync.dma_start(xt_sb[i][:, hs0], x_t[i][:, hs0])
        wk_sb, wg_sb, wv_sb = load_w(w_k), load_w(w_g), load_w(w_v)
        hs1 = slice(S // 2, S)
        for i in range(NCI):
            nc.sync.dma_start(xt_sb[i][:, hs1], x_t[i][:, hs1])
        bgv_sb = const.tile([P, 1], F32, tag="bgv")
        nc.sync.dma_start(bgv_sb[:], bg_t[:])
        mc_sb = const.tile([P, NKT], F32, tag="mc")
        nc.sync.dma_start(mc_sb[:], mc_t[:])
        wo_st = const.tile([P, C_IN], BF16, tag="wo")
        nc.sync.dma_start(wo_st[:], wo_t[:])

        from concourse.masks import make_identity

        ident = const.tile([P, P], BF16, tag="ident")
        make_identity(nc, ident[:])

        qT = const.tile([P, S], BF16, tag="qT")
        kT = const.tile([P, S], BF16, tag="kT")
        # head-swapped copies: rows [h1|h0]; lets a kt-pair run as two
        # concurrent row-group matmuls (K=64 each) in the 128-row PE array
        qTs = const.tile([P, S], BF16, tag="qTs")
        kTs = const.tile([P, S], BF16, tag="kTs")
        gT = [const.tile([C, S], BF16, tag=f"gT{h}", name=f"gT{h}") for h in range(2)]
        vm = [const.tile([P, 2 * (C + 1)], BF16, tag=f"vm{t}", name=f"vm{t}") for t in range(NKT)]
        goun2 = const.tile([P, S], BF16, tag="goun2")
        rrec = [const.tile([P, NKT], F32, tag=f"rrec{h}", name=f"rrec{h}") for h in range(2)]

        # ---------------- projections ----------------
        with tc.tile_pool(name="projp", bufs=2, space="PSUM") as projp:
            for wt, dest, dsw in ((wq_sb, qT, qTs), (wk_sb, kT, kTs), (wg_sb, None, None)):
                for ch in range(4):
                    pp = projp.tile([P, 512], F32, tag="pp")
                    sl = slice(ch * 512, (ch + 1) * 512)
                    for ci in range(NCI):
                        nc.tensor.matmul(
                            pp[:],
                            wt[:, ci, :],
                            xt_sb[ci][:, sl],
                            start=(ci == 0),
                            stop=(ci == NCI - 1),
                        )
                    if dest is None:
                        # gate via tanh: sigmoid(v) = 0.5 + 0.5*tanh(v/2).
                        # Tanh shares the ACT "exp_and_others" table set with
                        # Exp, so the kernel needs only ONE table load and the
                        # PE never idles through a mid-kernel table switch
                        # (a >3.4us PE stall re-throttles the HAM clock gate).
                        from concourse.alu_op_type import AluOpType as _Alu

                        for h in range(2):
                            hp = slice(C * h, C * (h + 1))
                            nc.scalar.activation(
                                gT[h][:, sl], pp[hp, :], Tanh,
                                bias=bgv_sb[hp, :], scale=0.5,
                            )
                            nc.vector.tensor_scalar(
                                gT[h][:, sl], gT[h][:, sl], 0.5, 0.5,
                                _Alu.mult, _Alu.add,
                            )
                    else:
                        nc.vector.tensor_copy(dest[:, sl], pp[:])
                        nc.vector.tensor_copy(dsw[0:C, sl], pp[C : 2 * C, :])
                        nc.vector.tensor_copy(dsw[C : 2 * C, sl], pp[0:C, :])

            for tt in range(NKT):
                pv = projp.tile([P, P], F32, tag="pv")
                for ci in range(NCI):
                    nc.tensor.matmul(
                        pv[:],
                        xt_sb[ci][:, tt * P : (tt + 1) * P],
                        wv_sb[:, ci, :],
                        start=(ci == 0),
                        stop=(ci == NCI - 1),
                    )
                mcol = mc_sb[:, tt : tt + 1]
                v = vm[tt]
                nc.vector.tensor_scalar_mul(v[:, 0:C], pv[:, 0:C], mcol)
                nc.vector.tensor_scalar_mul(v[:, C + 1 : 2 * C + 1], pv[:, C : 2 * C], mcol)
                nc.vector.tensor_copy(v[:, C : C + 1], mcol)
                nc.vector.tensor_copy(v[:, 2 * C + 1 : 2 * C + 2], mcol)

        # ---------------- attention per head ----------------
        with (
            tc.tile_pool(name="spsum", bufs=3, space="PSUM") as spsum,
            tc.tile_pool(name="opsum", bufs=1, space="PSUM") as opsum,
            tc.tile_pool(name="ebp", bufs=6) as ebp,
            tc.tile_pool(name="pexp", bufs=3) as pexp,
            tc.tile_pool(name="ptp", bufs=6) as ptp,
            tc.tile_pool(name="epi", bufs=2) as epi,
        ):
            # The pair bias is injected into PSUM with an identity matmul
            # (DMA cannot write PSUM) and the qk matmul accumulates on top;
            # exp then reads the finished logits straight out of PSUM. This
            # keeps the elementwise bias work on the PE (which would
            # otherwise sit half-idle behind ACT) instead of a 48us DVE
            # multiply, and the higher PE duty keeps the HAM clock-gate from
            # re-throttling the array to 1.2 GHz mid-phase.
            for h in range(2):
                hpA = slice(C * h, C * (h + 1))          # rows of qT/kT
                hpB = slice(C * (1 - h), C * (2 - h))    # rows of qTs/kTs
                for j in range(NQH):
                    jsl = slice(QH * j, QH * (j + 1))
                    op_ = opsum.tile([C + 1, QH], F32, tag="op")
                    for kp in range(NKT // 2):
                        k0, k1 = 2 * kp, 2 * kp + 1
                        ks0 = slice(k0 * P, (k0 + 1) * P)
                        ks1 = slice(k1 * P, (k1 + 1) * P)
                        ebA = ebp.tile([P, QH], BF16, tag="eb")
                        nc.sync.dma_start(ebA[:], eb_t[h, ks0, jsl])
                        ebB = ebp.tile([P, QH], BF16, tag="eb")
                        nc.sync.dma_start(ebB[:], eb_t[h, ks1, jsl])
                        spA = spsum.tile([P, QH], F32, tag="sp")
                        spB = spsum.tile([P, QH], F32, tag="sp")
                        # Even k-tile (A): bias injected on the PE via an
                        # identity matmul, exp reads finished logits from
                        # PSUM. Odd k-tile (B): plain qk matmul; the host
                        # pre-exponentiated that bias slice, applied as a
                        # DVE multiply after exp. Splitting the bias work
                        # between PE and DVE keeps both at ~the ACT exp
                        # rate, and the PE duty stays high enough that the
                        # HAM clock gate holds 2.4 GHz through the phase.
                        for ch in range(QH // 512):
                            csl = slice(ch * 512, (ch + 1) * 512)
                            nc.tensor.matmul(
                                spA[:, csl], ident[:], ebA[:, csl],
                                start=True, stop=False,
                            )
                        for ch in range(QH // 512):
                            csl = slice(ch * 512, (ch + 1) * 512)
                            qs = QH * j + 512 * ch
                            nc.tensor.matmul(
                                spA[:, csl], kT[hpA, ks0],
                                qT[hpA, qs : qs + 512],
                                start=False, stop=True,
                            )
                            nc.tensor.matmul(
                                spB[:, csl], kTs[hpB, ks1],
                                qTs[hpB, qs : qs + 512],
                                start=True, stop=True,
                            )
                        ptA = ptp.tile([P, QH], BF16, tag="pt")
                        nc.scalar.activation(ptA[:], spA[:], Exp)
                        exB = pexp.tile([P, QH], BF16, tag="ex")
                        nc.scalar.activation(exB[:], spB[:], Exp)
                        ptB = ptp.tile([P, QH], BF16, tag="pt")
                        nc.vector.tensor_mul(ptB[:], exB[:], ebB[:])
                        for kt, pt in ((k0, ptA), (k1, ptB)):
                            for ch in range(QH // 512):
                                qs = 512 * ch
                                nc.tensor.matmul(
                                    op_[:, qs : qs + 512],
                                    vm[kt][:, (C + 1) * h : (C + 1) * (h + 1)],
                                    pt[:, qs : qs + 512],
                                    start=(kt == 0),
                                    stop=(kt == NKT - 1),
                                )

                    # epilogue for this q-half: rowsum -> per-q-tile columns
                    # (DRAM round trip; a single-row reciprocal would run on
                    # one DVE lane = ~13us), reciprocal on 128 lanes, gate
                    # applied unnormalized. The 1/rowsum scale is applied
                    # after the output projection (per-partition scalar
                    # there, since q sits on partitions).
                    rsum = epi.tile([1, QH], F32, tag="rsum")
                    nc.vector.tensor_copy(rsum[:], op_[C : C + 1, :])
                    dscr = drp.tile([1, QH], F32, tag="dscr")
                    nc.sync.dma_start(dscr[:], rsum[:])
                    rsl = slice((QH // P) * j, (QH // P) * (j + 1))
                    nc.sync.dma_start(
                        rrec[h][:, rsl],
                        dscr[0, :].rearrange("(t p) -> p t", p=P),
                    )
                    nc.vector.reciprocal(rrec[h][:, rsl], rrec[h][:, rsl])
                    nc.vector.tensor_mul(
                        goun2[C * h : C * (h + 1), jsl], op_[0:C, :], gT[h][:, jsl]
                    )

        # ---------------- output projection ----------------
        with (
            tc.tile_pool(name="outp", bufs=2, space="PSUM") as outp,
            tc.tile_pool(name="outs", bufs=3) as outs,
        ):
            from concourse.alu_op_type import AluOpType

            for qt in range(NKT):
                qsl = slice(qt * P, (qt + 1) * P)
                po0 = outp.tile([P, C_IN], F32, tag="po0")
                nc.tensor.matmul(po0[:], goun2[0:C, qsl], wo_st[0:C, :],
                                 start=True, stop=True)
                po1 = outp.tile([P, C_IN], F32, tag="po1")
                nc.tensor.matmul(po1[:], goun2[C:P, qsl], wo_st[C:P, :],
                                 start=True, stop=True)
                t1 = outs.tile([P, C_IN], F32, tag="t1")
                nc.scalar.activation(t1[:], po0[:], mybir.ActivationFunctionType.Copy,
                                     scale=rrec[0][:, qt : qt + 1])
                ob = outs.tile([P, C_IN], F32, tag="ob")
                nc.vector.scalar_tensor_tensor(
                    ob[:], po1[:], rrec[1][:, qt : qt + 1], t1[:],
                    AluOpType.mult, AluOpType.add,
                )
                nc.sync.dma_start(out_t[qsl, :], ob[:])

    return nc


_NC_CACHE = None


def _get_nc():
    global _NC_CACHE
    if _NC_CACHE is None:
        _NC_CACHE = _build_nc()
    return _NC_CACHE


def _prepare_core(c, x, bias, attention_mask, Wq, Wk, Wv, Wg, bg, Wo):
    b = c // 4
    h1 = 2 * (c % 4)
    h2 = h1 + 1
    sl1 = slice(h1 * C, (h1 + 1) * C)
    sl2 = slice(h2 * C, (h2 + 1) * C)

    xt = np.ascontiguousarray(x[b].T).reshape(NCI, P, S).astype(NBF)

    def wsel(W, scale=1.0):
        w = np.concatenate([W[:, sl1], W[:, sl2]], axis=1)
        if scale != 1.0:
            w = w * scale
        return np.ascontiguousarray(w.reshape(NCI, P, P)).astype(NBF)

    wq = wsel(Wq, 1.0 / np.sqrt(C))
    wk = wsel(Wk)
    wg = wsel(Wg)
    wv = wsel(Wv)
    bgv = (0.5 * np.concatenate([bg[sl1], bg[sl2]])).reshape(P, 1).astype(np.float32)
    maskcol = np.ascontiguousarray(
        attention_mask[b].astype(np.float32).reshape(NKT, P).T
    )
    # transposed pair bias: ebias[j, k, q] = bias[b, h_j, q, k]; odd
    # k-tiles are pre-exponentiated (device applies them as exp(qk)*exp(b))
    eb = np.empty((2, S, S), dtype=NBF)
    for j, hh in enumerate((h1, h2)):
        bt = np.ascontiguousarray(bias[b, hh].T)
        bt4 = bt.reshape(NKT, P, S)
        for kt in range(NKT):
            if kt % 2:
                eb[j, kt * P : (kt + 1) * P] = np.exp(bt4[kt]).astype(NBF)
            else:
                eb[j, kt * P : (kt + 1) * P] = bt4[kt].astype(NBF)
    wo = np.concatenate([Wo[sl1, :], Wo[sl2, :]], 0).astype(NBF)

    return {
        "xt": xt,
        "wq": wq,
        "wk": wk,
        "wg": wg,
        "wv": wv,
        "bgv": bgv,
        "maskcol": maskcol,
        "ebias": eb,
        "wo": wo,
    }


def _run(inputs, trace=False, **kw):
    x = np.asarray(inputs["x"], dtype=np.float32)
    bias = np.asarray(inputs["bias"], dtype=np.float32)
    attention_mask = np.asarray(inputs["attention_mask"])
    Wq = np.asarray(inputs["Wq"], dtype=np.float32)
    Wk = np.asarray(inputs["Wk"], dtype=np.float32)
    Wv = np.asarray(inputs["Wv"], dtype=np.float32)
    Wg = np.asarray(inputs["Wg"], dtype=np.float32)
    bg = np.asarray(inputs["bg"], dtype=np.float32)
    Wo = np.asarray(inputs["Wo"], dtype=np.float32)
    bo = np.asarray(inputs["bo"], dtype=np.float32)

    in_maps = [None] * 8

    def prep(c):
        in_maps[c] = _prepare_core(c, x, bias, attention_mask, Wq, Wk, Wv, Wg, bg, Wo)

    threads = [threading.Thread(target=prep, args=(c,)) for c in range(8)]
    for t in threads:
        t.start()
    for t in threads:
        t.join()

    nc = _get_nc()
    res = run_bass_kernel_spmd(nc, in_maps, core_ids=list(range(8)), trace=trace, **kw)

    out = np.empty((B, S, C_IN), dtype=np.float32)
    for b in range(B):
        acc = res.results[4 * b]["out"].astype(np.float32)
        for c in range(4 * b + 1, 4 * b + 4):
            acc = acc + res.results[c]["out"]
        out[b] = acc + bo[None, :]
    return out, res


def kernel(**inputs) -> np.ndarray:
    return _run(inputs)[0]


# revision 17
# speedup vs baseline: 1.1188x; 1.1188x over previous
"""Trainium2 Bass kernel for gated multi-head attention with pair bias.

Reference computation (B=2, S=2048, C_IN=512, H=8, C=64):
    q,k,v = heads(x @ Wq), heads(x @ Wk), heads(x @ Wv)
    logits = q k^T / sqrt(C) + bias + mask_offset
    attn   = softmax(logits)
    o      = attn @ v
    out    = (sigmoid(x @ Wg + bg) * concat(o)) @ Wo + bo

Sharding: 8 cores = 2 batches x 4 head-pairs. Core c handles batch c//4,
heads (2*(c%4), 2*(c%4)+1). Weights are sliced per-core on the host; each
core computes a partial output (sum over its two heads) and the host sums
4 partials per batch and adds bo.

Device math notes:
  - Everything is computed in the "transposed" orientation (feature dim on
    partitions) so the PE contraction dims line up without any on-device
    transposes: qT/kT/gT = W^T x^T via matmul(lhsT=W_chunk, rhs=xT_chunk).
  - softmax skips the max-subtraction (logits are O(+-8): exp is safe in
    fp32) and uses exp(qk) * exp(bias), with exp(bias^T) precomputed on the
    host in bf16. The key mask is folded into v (v*m) and the softmax
    denominator comes from an extra ones*m column appended to v, so the
    attention matmul produces [o_unnorm ; rowsum] in one accumulation.
  - All matmuls run in bf16 (fp32 PSUM accumulation).
"""

import os
import sys
import threading

import numpy as np

sys.path.insert(0, "/opt/trn_rl_repo")

import ml_dtypes

import concourse.bass as bass
import concourse.tile as tile
from concourse import mybir
from concourse.bass_utils import run_bass_kernel_spmd

# ---------------------------------------------------------------------------
# This toolchain's walrus encodes at most ONE semaphore wait per Drain/CTRL
# instruction; Tile's end-of-kernel drain can carry several (one per DMA
# queue). Split them across a chain of single-wait drains.
# ---------------------------------------------------------------------------


_NOP_UID = [0]


def _split_multi_waits(nc):
    """Rewrite every instruction carrying >1 sem waits: keep one wait on the
    instruction, hoist the others onto same-engine NoOps inserted right
    before it (engine streams execute in order, so this is equivalent)."""
    for fn in nc.m.functions:
        for bb in fn.blocks:
            insts = list(bb.instructions)
            out = []
            changed = False
            for inst in insts:
                si = inst.sync_info
                if si is not None and len(si.on_wait) > 1:
                    changed = True
                    waits = list(si.on_wait)
                    si.on_wait = waits[:1]
                    for w in waits[1:]:
                        _NOP_UID[0] += 1
                        nop = mybir.InstNoOp(
                            name=f"waitsplit-{_NOP_UID[0]}",
                            engine=inst.engine,
                            ins=[],
                            outs=[],
                        )
                        nop.sync_info = mybir.SyncInfo(on_wait=[w], on_update=[])
                        out.append(nop)
                out.append(inst)
            if changed:
                bb.instructions = out


def _drain_and_barrier_split(self, tick_clock, wait_clock):
    from concourse.vector_clock import ScopedClock

    drain_inst = self.nc.sync.drain()
    wait_clock.add_sem_waits(
        drain_inst.ins, ScopedClock({None: tick_clock.global_clock})
    )
    si = drain_inst.ins.sync_info
    if si is not None and len(si.on_wait) > 1:
        extra = list(si.on_wait[1:])
        si.on_wait = list(si.on_wait[:1])
        for w in extra:
            d2 = self.nc.sync.drain()
            d2.ins.sync_info = mybir.SyncInfo(on_wait=[w], on_update=[])

    self.nc.all_engine_barrier()
    assert self.sems is not None
    popped = self.nc._tile_sem_poison_stack.pop()
    assert popped is self._sem_poison
    self.nc.clear_and_free_semaphores(list(self.sems.allocated().values()))
    self.nc.all_engine_barrier()

    _split_multi_waits(self.nc)


tile.TileContext._drain_and_barrier = _drain_and_barrier_split

BF16 = mybir.dt.bfloat16
F32 = mybir.dt.float32
NBF = ml_dtypes.bfloat16

B, S, C_IN, H, C = 2, 2048, 512, 8, 64
P = 128
NKT = S // P  # 16 key tiles
NCI = C_IN // P  # 4 contraction chunks
QH = 1024  # q-chunk processed per exp instruction
NQH = S // QH  # 2

Exp = mybir.ActivationFunctionType.Exp
Tanh = mybir.ActivationFunctionType.Tanh


def _build_nc():
    nc = bass.Bass("TRN2")

    x_t = nc.dram_tensor("xt", [NCI, P, S], BF16, kind="ExternalInput")
    w_q = nc.dram_tensor("wq", [NCI, P, P], BF16, kind="ExternalInput")
    w_k = nc.dram_tensor("wk", [NCI, P, P], BF16, kind="ExternalInput")
    w_g = nc.dram_tensor("wg", [NCI, P, P], BF16, kind="ExternalInput")
    w_v = nc.dram_tensor("wv", [NCI, P, P], BF16, kind="ExternalInput")
    bg_t = nc.dram_tensor("bgv", [P, 1], F32, kind="ExternalInput")
    mc_t = nc.dram_tensor("maskcol", [P, NKT], F32, kind="ExternalInput")
    eb_t = nc.dram_tensor("ebias", [2, S, S], BF16, kind="ExternalInput")
    wo_t = nc.dram_tensor("wo", [P, C_IN], BF16, kind="ExternalInput")
    out_t = nc.dram_tensor("out", [S, C_IN], F32, kind="ExternalOutput")

    from contextlib import ExitStack

    with tile.TileContext(nc) as tc, ExitStack() as ctx:
        const = ctx.enter_context(tc.tile_pool(name="const", bufs=1))
        drp = ctx.enter_context(tc.tile_pool(name="dram", bufs=2, space="DRAM"))

        # ---------------- persistent SBUF tiles ----------------
        # weights first (tiny), then x^T in column halves so the first
        # projection chunks can start before the whole input has landed
        def load_w(dram):
            t = const.tile([P, NCI, P], BF16, tag=f"w{dram.name}", name=f"wsb_{dram.name}")
            nc.sync.dma_start(t[:], dram[:].rearrange("c p m -> p c m"))
            return t

        wq_sb, wk_sb, wg_sb, wv_sb = load_w(w_q), load_w(w_k), load_w(w_g), load_w(w_v)
        xt_sb = [const.tile([P, S], BF16, tag=f"xt{i}", name=f"xtsb{i}") for i in range(NCI)]
        for half in range(2):
            hs = slice(half * (S // 2), (half + 1) * (S // 2))
            for i in range(NCI):
                nc.sync.dma_start(xt_sb[i][:, hs], x_t[i][:, hs])
        bgv_sb = const.tile([P, 1], F32, tag="bgv")
        nc.sync.dma_start(bgv_sb[:], bg_t[:])
        mc_sb = const.tile([P, NKT], F32, tag="mc")
        nc.sync.dma_start(mc_sb[:], mc_t[:])
        wo_st = const.tile([P, C_IN], BF16, tag="wo")
        nc.sync.dma_start(wo_st[:], wo_t[:])

        from concourse.masks import make_identity

        ident = const.tile([P, P], BF16, tag="ident")
        make_identity(nc, ident[:])

        qT = const.tile([P, S], BF16, tag="qT")
        kT = const.tile([P, S], BF16, tag="kT")
        # head-swapped copies: rows [h1|h0]; lets a kt-pair run as two
        # concurrent row-group matmuls (K=64 each) in the 128-row PE array
        qTs = const.tile([P, S], BF16, tag="qTs")
        kTs = const.tile([P, S], BF16, tag="kTs")
        gT = [const.tile([C, S], BF16, tag=f"gT{h}", name=f"gT{h}") for h in range(2)]
        vm = [const.tile([P, 2 * (C + 1)], BF16, tag=f"vm{t}", name=f"vm{t}") for t in range(NKT)]
        goun2 = const.tile([P, S], BF16, tag="goun2")
        rrec = [const.tile([P, NKT], F32, tag=f"rrec{h}", name=f"rrec{h}") for h in range(2)]

        # ---------------- projections ----------------
        with tc.tile_pool(name="projp", bufs=2, space="PSUM") as projp:
            for wt, dest, dsw in ((wq_sb, qT, qTs), (wk_sb, kT, kTs), (wg_sb, None, None)):
                for ch in range(4):
                    pp = projp.tile([P, 512], F32, tag="pp")
                    sl = slice(ch * 512, (ch + 1) * 512)
                    for ci in range(NCI):
                        nc.tensor.matmul(
                            pp[:],
                            wt[:, ci, :],
                            xt_sb[ci][:, sl],
                            start=(ci == 0),
                            stop=(ci == NCI - 1),
                        )
                    if dest is None:
                        # gate via tanh: sigmoid(v) = 0.5 + 0.5*tanh(v/2).
                        # Tanh shares the ACT "exp_and_others" table set with
                        # Exp, so the kernel needs only ONE table load and the
                        # PE never idles through a mid-kernel table switch
                        # (a >3.4us PE stall re-throttles the HAM clock gate).
                        from concourse.alu_op_type import AluOpType as _Alu

                        for h in range(2):
                            hp = slice(C * h, C * (h + 1))
                            nc.scalar.activation(
                                gT[h][:, sl], pp[hp, :], Tanh,
                                bias=bgv_sb[hp, :], scale=0.5,
                            )
                            nc.vector.tensor_scalar(
                                gT[h][:, sl], gT[h][:, sl], 0.5, 0.5,
                                _Alu.mult, _Alu.add,
                            )
                    else:
                        nc.vector.tensor_copy(dest[:, sl], pp[:])
                        nc.vector.tensor_copy(dsw[0:C, sl], pp[C : 2 * C, :])
                        nc.vector.tensor_copy(dsw[C : 2 * C, sl], pp[0:C, :])

            for tt in range(NKT):
                pv = projp.tile([P, P], F32, tag="pv")
                for ci in range(NCI):
                    nc.tensor.matmul(
                        pv[:],
                        xt_sb[ci][:, tt * P : (tt + 1) * P],
                        wv_sb[:, ci, :],
                        start=(ci == 0),
                        stop=(ci == NCI - 1),
                    )
                mcol = mc_sb[:, tt : tt + 1]
                v = vm[tt]
                nc.vector.tensor_scalar_mul(v[:, 0:C], pv[:, 0:C], mcol)
                nc.vector.tensor_scalar_mul(v[:, C + 1 : 2 * C + 1], pv[:, C : 2 * C], mcol)
                nc.vector.tensor_copy(v[:, C : C + 1], mcol)
                nc.vector.tensor_copy(v[:, 2 * C + 1 : 2 * C + 2], mcol)

        # ---------------- attention per head ----------------
        with (
            tc.tile_pool(name="spsum", bufs=3, space="PSUM") as spsum,
            tc.tile_pool(name="opsum", bufs=1, space="PSUM") as opsum,
            tc.tile_pool(name="ebp", bufs=6) as ebp,
            tc.tile_pool(name="pexp", bufs=3) as pexp,
            tc.tile_pool(name="ptp", bufs=6) as ptp,
            tc.tile_pool(name="epi", bufs=2) as epi,
        ):
            # The pair bias is injected into PSUM with an identity matmul
            # (DMA cannot write PSUM) and the qk matmul accumulates on top;
            # exp then reads the finished logits straight out of PSUM. This
            # keeps the elementwise bias work on the PE (which would
            # otherwise sit half-idle behind ACT) instead of a 48us DVE
            # multiply, and the higher PE duty keeps the HAM clock-gate from
            # re-throttling the array to 1.2 GHz mid-phase.
            for h in range(2):
                hpA = slice(C * h, C * (h + 1))          # rows of qT/kT
                hpB = slice(C * (1 - h), C * (2 - h))    # rows of qTs/kTs
                for j in range(NQH):
                    jsl = slice(QH * j, QH * (j + 1))
                    op_ = opsum.tile([C + 1, QH], F32, tag="op")
                    for kp in range(NKT // 2):
                        k0, k1 = 2 * kp, 2 * kp + 1
                        ks0 = slice(k0 * P, (k0 + 1) * P)
                        ks1 = slice(k1 * P, (k1 + 1) * P)
                        ebA = ebp.tile([P, QH], BF16, tag="eb")
                        nc.sync.dma_start(ebA[:], eb_t[h, ks0, jsl])
                        ebB = ebp.tile([P, QH], BF16, tag="eb")
                        nc.sync.dma_start(ebB[:], eb_t[h, ks1, jsl])
                        spA = spsum.tile([P, QH], F32, tag="sp")
                        spB = spsum.tile([P, QH], F32, tag="sp")
                        # Even k-tile (A): bias injected on the PE via an
                        # identity matmul, exp reads finished logits from
                        # PSUM. Odd k-tile (B): plain qk matmul; the host
                        # pre-exponentiated that bias slice, applied as a
                        # DVE multiply after exp. Splitting the bias work
                        # between PE and DVE keeps both at ~the ACT exp
                        # rate, and the PE duty stays high enough that the
                        # HAM clock gate holds 2.4 GHz through the phase.
                        for ch in range(QH // 512):
                            csl = slice(ch * 512, (ch + 1) * 512)
                            nc.tensor.matmul(
                                spA[:, csl], ident[:], ebA[:, csl],
                                start=True, stop=False,
                            )
                        for ch in range(QH // 512):
                            csl = slice(ch * 512, (ch + 1) * 512)
                            qs = QH * j + 512 * ch
                            nc.tensor.matmul(
                                spA[:, csl], kT[hpA, ks0],
                                qT[hpA, qs : qs + 512],
                                start=False, stop=True,
                            )
                            nc.tensor.matmul(
                                spB[:, csl], kTs[hpB, ks1],
                                qTs[hpB, qs : qs + 512],
                                start=True, stop=True,
                            )
                        ptA = ptp.tile([P, QH], BF16, tag="pt")
                        nc.scalar.activation(ptA[:], spA[:], Exp)
                        exB = pexp.tile([P, QH], BF16, tag="ex")
                        nc.scalar.activation(exB[:], spB[:], Exp)
                        ptB = ptp.tile([P, QH], BF16, tag="pt")
                        nc.vector.tensor_mul(ptB[:], exB[:], ebB[:])
                        for kt, pt in ((k0, ptA), (k1, ptB)):
                            for ch in range(QH // 512):
                                qs = 512 * ch
                                nc.tensor.matmul(
                                    op_[:, qs : qs + 512],
                                    vm[kt][:, (C + 1) * h : (C + 1) * (h + 1)],
                                    pt[:, qs : qs + 512],
                                    start=(kt == 0),
                                    stop=(kt == NKT - 1),
                                )

                    # epilogue for this q-half: rowsum -> per-q-tile columns
                    # (DRAM round trip; a single-row reciprocal would run on
                    # one DVE lane = ~13us), reciprocal on 128 lanes, gate
                    # applied unnormalized. The 1/rowsum scale is applied
                    # after the output projection (per-partition scalar
                    # there, since q sits on partitions).
                    rsum = epi.tile([1, QH], F32, tag="rsum")
                    nc.vector.tensor_copy(rsum[:], op_[C : C + 1, :])
                    dscr = drp.tile([1, QH], F32, tag="dscr")
                    nc.sync.dma_start(dscr[:], rsum[:])
                    nc.sync.dma_start(
                        rrec[h][:, (QH // P) * j : (QH // P) * (j + 1)],
                        dscr[0, :].rearrange("(t p) -> p t", p=P),
                    )
                    nc.vector.tensor_mul(
                        goun2[C * h : C * (h + 1), jsl], op_[0:C, :], gT[h][:, jsl]
                    )
            nc.vector.reciprocal(rrec[0][:], rrec[0][:])
            nc.vector.reciprocal(rrec[1][:], rrec[1][:])

        # ---------------- output projection ----------------
        with (
            tc.tile_pool(name="outp", bufs=2, space="PSUM") as outp,
            tc.tile_pool(name="outs", bufs=3) as outs,
        ):
            from concourse.alu_op_type import AluOpType

            for qt in range(NKT):
                qsl = slice(qt * P, (qt + 1) * P)
                po0 = outp.tile([P, C_IN], F32, tag="po0")
                nc.tensor.matmul(po0[:], goun2[0:C, qsl], wo_st[0:C, :],
                                 start=True, stop=True)
                po1 = outp.tile([P, C_IN], F32, tag="po1")
                nc.tensor.matmul(po1[:], goun2[C:P, qsl], wo_st[C:P, :],
                                 start=True, stop=True)
                t1 = outs.tile([P, C_IN], F32, tag="t1")
                nc.vector.tensor_scalar_mul(t1[:], po0[:], rrec[0][:, qt : qt + 1])
                ob = outs.tile([P, C_IN], F32, tag="ob")
                nc.vector.scalar_tensor_tensor(
                    ob[:], po1[:], rrec[1][:, qt : qt + 1], t1[:],
                    AluOpType.mult, AluOpType.add,
                )
                nc.sync.dma_start(out_t[qsl, :], ob[:])

    return nc


_NC_CACHE = None


def _get_nc():
    global _NC_CACHE
    if _NC_CACHE is None:
        _NC_CACHE = _build_nc()
    return _NC_CACHE


def _prepare_core(c, x, bias, attention_mask, Wq, Wk, Wv, Wg, bg, Wo):
    b = c // 4
    h1 = 2 * (c % 4)
    h2 = h1 + 1
    sl1 = slice(h1 * C, (h1 + 1) * C)
    sl2 = slice(h2 * C, (h2 + 1) * C)

    xt = np.ascontiguousarray(x[b].T).reshape(NCI, P, S).astype(NBF)

    def wsel(W, scale=1.0):
        w = np.concatenate([W[:, sl1], W[:, sl2]], axis=1)
        if scale != 1.0:
            w = w * scale
        return np.ascontiguousarray(w.reshape(NCI, P, P)).astype(NBF)

    wq = wsel(Wq, 1.0 / np.sqrt(C))
    wk = wsel(Wk)
    wg = wsel(Wg)
    wv = wsel(Wv)
    bgv = (0.5 * np.concatenate([bg[sl1], bg[sl2]])).reshape(P, 1).astype(np.float32)
    maskcol = np.ascontiguousarray(
        attention_mask[b].astype(np.float32).reshape(NKT, P).T
    )
    # transposed pair bias: ebias[j, k, q] = bias[b, h_j, q, k]; odd
    # k-tiles are pre-exponentiated (device applies them as exp(qk)*exp(b))
    eb = np.empty((2, S, S), dtype=NBF)
    for j, hh in enumerate((h1, h2)):
        bt = np.ascontiguousarray(bias[b, hh].T)
        bt4 = bt.reshape(NKT, P, S)
        for kt in range(NKT):
            if kt % 2:
                eb[j, kt * P : (kt + 1) * P] = np.exp(bt4[kt]).astype(NBF)
            else:
                eb[j, kt * P : (kt + 1) * P] = bt4[kt].astype(NBF)
    wo = np.concatenate([Wo[sl1, :], Wo[sl2, :]], 0).astype(NBF)

    return {
        "xt": xt,
        "wq": wq,
        "wk": wk,
        "wg": wg,
        "wv": wv,
        "bgv": bgv,
        "maskcol": maskcol,
        "ebias": eb,
        "wo": wo,
    }


def _run(inputs, trace=False, **kw):
    x = np.asarray(inputs["x"], dtype=np.float32)
    bias = np.asarray(inputs["bias"], dtype=np.float32)
    attention_mask = np.asarray(inputs["attention_mask"])
    Wq = np.asarray(inputs["Wq"], dtype=np.float32)
    Wk = np.asarray(inputs["Wk"], dtype=np.float32)
    Wv = np.asarray(inputs["Wv"], dtype=np.float32)
    Wg = np.asarray(inputs["Wg"], dtype=np.float32)
    bg = np.asarray(inputs["bg"], dtype=np.float32)
    Wo = np.asarray(inputs["Wo"], dtype=np.float32)
    bo = np.asarray(inputs["bo"], dtype=np.float32)

    in_maps = [None] * 8

    def prep(c):
        in_maps[c] = _prepare_core(c, x, bias, attention_mask, Wq, Wk, Wv, Wg, bg, Wo)

    threads = [threading.Thread(target=prep, args=(c,)) for c in range(8)]
    for t in threads:
        t.start()
    for t in threads:
        t.join()

    nc = _get_nc()
    res = run_bass_kernel_spmd(nc, in_maps, core_ids=list(range(8)), trace=trace, **kw)

    out = np.empty((B, S, C_IN), dtype=np.float32)
    for b in range(B):
        acc = res.results[4 * b]["out"].astype(np.float32)
        for c in range(4 * b + 1, 4 * b + 4):
            acc = acc + res.results[c]["out"]
        out[b] = acc + bo[None, :]
    return out, res


def kernel(**inputs) -> np.ndarray:
    return _run(inputs)[0]


# revision 18
# speedup vs baseline: 1.1253x; 1.0058x over previous
"""Trainium2 Bass kernel for gated multi-head attention with pair bias.

Reference computation (B=2, S=2048, C_IN=512, H=8, C=64):
    q,k,v = heads(x @ Wq), heads(x @ Wk), heads(x @ Wv)
    logits = q k^T / sqrt(C) + bias + mask_offset
    attn   = softmax(logits)
    o      = attn @ v
    out    = (sigmoid(x @ Wg + bg) * concat(o)) @ Wo + bo

Sharding: 8 cores = 2 batches x 4 head-pairs. Core c handles batch c//4,
heads (2*(c%4), 2*(c%4)+1). Weights are sliced per-core on the host; each
core computes a partial output (sum over its two heads) and the host sums
4 partials per batch and adds bo.

Device math notes:
  - Everything is computed in the "transposed" orientation (feature dim on
    partitions) so the PE contraction dims line up without any on-device
    transposes: qT/kT/gT = W^T x^T via matmul(lhsT=W_chunk, rhs=xT_chunk).
  - softmax skips the max-subtraction (logits are O(+-8): exp is safe in
    fp32) and uses exp(qk) * exp(bias), with exp(bias^T) precomputed on the
    host in bf16. The key mask is folded into v (v*m) and the softmax
    denominator comes from an extra ones*m column appended to v, so the
    attention matmul produces [o_unnorm ; rowsum] in one accumulation.
  - All matmuls run in bf16 (fp32 PSUM accumulation).
"""

import os
import sys
import threading

import numpy as np

sys.path.insert(0, "/opt/trn_rl_repo")

import ml_dtypes

import concourse.bass as bass
import concourse.tile as tile
from concourse import mybir
from concourse.bass_utils import run_bass_kernel_spmd

# ---------------------------------------------------------------------------
# This toolchain's walrus encodes at most ONE semaphore wait per Drain/CTRL
# instruction; Tile's end-of-kernel drain can carry several (one per DMA
# queue). Split them across a chain of single-wait drains.
# ---------------------------------------------------------------------------


_NOP_UID = [0]


def _split_multi_waits(nc):
    """Rewrite every instruction carrying >1 sem waits: keep one wait on the
    instruction, hoist the others onto same-engine NoOps inserted right
    before it (engine streams execute in order, so this is equivalent)."""
    for fn in nc.m.functions:
        for bb in fn.blocks:
            insts = list(bb.instructions)
            out = []
            changed = False
            for inst in insts:
                si = inst.sync_info
                if si is not None and len(si.on_wait) > 1:
                    changed = True
                    waits = list(si.on_wait)
                    si.on_wait = waits[:1]
                    for w in waits[1:]:
                        _NOP_UID[0] += 1
                        nop = mybir.InstNoOp(
                            name=f"waitsplit-{_NOP_UID[0]}",
                            engine=inst.engine,
                            ins=[],
                            outs=[],
                        )
                        nop.sync_info = mybir.SyncInfo(on_wait=[w], on_update=[])
                        out.append(nop)
                out.append(inst)
            if changed:
                bb.instructions = out


def _drain_and_barrier_split(self, tick_clock, wait_clock):
    from concourse.vector_clock import ScopedClock

    drain_inst = self.nc.sync.drain()
    wait_clock.add_sem_waits(
        drain_inst.ins, ScopedClock({None: tick_clock.global_clock})
    )
    si = drain_inst.ins.sync_info
    if si is not None and len(si.on_wait) > 1:
        extra = list(si.on_wait[1:])
        si.on_wait = list(si.on_wait[:1])
        for w in extra:
            d2 = self.nc.sync.drain()
            d2.ins.sync_info = mybir.SyncInfo(on_wait=[w], on_update=[])

    self.nc.all_engine_barrier()
    assert self.sems is not None
    popped = self.nc._tile_sem_poison_stack.pop()
    assert popped is self._sem_poison
    self.nc.clear_and_free_semaphores(list(self.sems.allocated().values()))
    self.nc.all_engine_barrier()

    _split_multi_waits(self.nc)


tile.TileContext._drain_and_barrier = _drain_and_barrier_split

BF16 = mybir.dt.bfloat16
F32 = mybir.dt.float32
NBF = ml_dtypes.bfloat16

B, S, C_IN, H, C = 2, 2048, 512, 8, 64
P = 128
NKT = S // P  # 16 key tiles
NCI = C_IN // P  # 4 contraction chunks
QH = 1024  # q-chunk processed per exp instruction
NQH = S // QH  # 2

Exp = mybir.ActivationFunctionType.Exp
Tanh = mybir.ActivationFunctionType.Tanh


def _build_nc():
    nc = bass.Bass("TRN2")

    x_t = nc.dram_tensor("xt", [NCI, P, S], BF16, kind="ExternalInput")
    w_q = nc.dram_tensor("wq", [NCI, P, P], BF16, kind="ExternalInput")
    w_k = nc.dram_tensor("wk", [NCI, P, P], BF16, kind="ExternalInput")
    w_g = nc.dram_tensor("wg", [NCI, P, P], BF16, kind="ExternalInput")
    w_v = nc.dram_tensor("wv", [NCI, P, P], BF16, kind="ExternalInput")
    bg_t = nc.dram_tensor("bgv", [P, 1], F32, kind="ExternalInput")
    mc_t = nc.dram_tensor("maskcol", [P, NKT], F32, kind="ExternalInput")
    eb_t = nc.dram_tensor("ebias", [2, S, S], BF16, kind="ExternalInput")
    wo_t = nc.dram_tensor("wo", [P, C_IN], BF16, kind="ExternalInput")
    out_t = nc.dram_tensor("out", [S, C_IN], F32, kind="ExternalOutput")

    from contextlib import ExitStack

    with tile.TileContext(nc) as tc, ExitStack() as ctx:
        const = ctx.enter_context(tc.tile_pool(name="const", bufs=1))
        drp = ctx.enter_context(tc.tile_pool(name="dram", bufs=2, space="DRAM"))

        # ---------------- persistent SBUF tiles ----------------
        # weights first (tiny), then x^T in column halves so the first
        # projection chunks can start before the whole input has landed
        def load_w(dram):
            t = const.tile([P, NCI, P], BF16, tag=f"w{dram.name}", name=f"wsb_{dram.name}")
            nc.sync.dma_start(t[:], dram[:].rearrange("c p m -> p c m"))
            return t

        wq_sb, wk_sb, wg_sb, wv_sb = load_w(w_q), load_w(w_k), load_w(w_g), load_w(w_v)
        xt_sb = [const.tile([P, S], BF16, tag=f"xt{i}", name=f"xtsb{i}") for i in range(NCI)]
        for half in range(2):
            hs = slice(half * (S // 2), (half + 1) * (S // 2))
            for i in range(NCI):
                nc.sync.dma_start(xt_sb[i][:, hs], x_t[i][:, hs])
        bgv_sb = const.tile([P, 1], F32, tag="bgv")
        nc.sync.dma_start(bgv_sb[:], bg_t[:])
        mc_sb = const.tile([P, NKT], F32, tag="mc")
        nc.sync.dma_start(mc_sb[:], mc_t[:])
        wo_st = const.tile([P, C_IN], BF16, tag="wo")
        nc.sync.dma_start(wo_st[:], wo_t[:])

        from concourse.masks import make_identity

        ident = const.tile([P, P], BF16, tag="ident")
        make_identity(nc, ident[:])

        qT = const.tile([P, S], BF16, tag="qT")
        kT = const.tile([P, S], BF16, tag="kT")
        # head-swapped copies: rows [h1|h0]; lets a kt-pair run as two
        # concurrent row-group matmuls (K=64 each) in the 128-row PE array
        qTs = const.tile([P, S], BF16, tag="qTs")
        kTs = const.tile([P, S], BF16, tag="kTs")
        gT = [const.tile([C, S], BF16, tag=f"gT{h}", name=f"gT{h}") for h in range(2)]
        vm = [const.tile([P, 2 * (C + 1)], BF16, tag=f"vm{t}", name=f"vm{t}") for t in range(NKT)]
        goun2 = const.tile([P, S], BF16, tag="goun2")
        rrec = [const.tile([P, NKT], F32, tag=f"rrec{h}", name=f"rrec{h}") for h in range(2)]

        # ---------------- projections ----------------
        with tc.tile_pool(name="projp", bufs=2, space="PSUM") as projp:
            for wt, dest, dsw in ((wq_sb, qT, qTs), (wk_sb, kT, kTs), (wg_sb, None, None)):
                for ch in range(4):
                    pp = projp.tile([P, 512], F32, tag="pp")
                    sl = slice(ch * 512, (ch + 1) * 512)
                    for ci in range(NCI):
                        nc.tensor.matmul(
                            pp[:],
                            wt[:, ci, :],
                            xt_sb[ci][:, sl],
                            start=(ci == 0),
                            stop=(ci == NCI - 1),
                        )
                    if dest is None:
                        # gate via tanh: sigmoid(v) = 0.5 + 0.5*tanh(v/2).
                        # Tanh shares the ACT "exp_and_others" table set with
                        # Exp, so the kernel needs only ONE table load and the
                        # PE never idles through a mid-kernel table switch
                        # (a >3.4us PE stall re-throttles the HAM clock gate).
                        from concourse.alu_op_type import AluOpType as _Alu

                        for h in range(2):
                            hp = slice(C * h, C * (h + 1))
                            nc.scalar.activation(
                                gT[h][:, sl], pp[hp, :], Tanh,
                                bias=bgv_sb[hp, :], scale=0.5,
                            )
                            nc.vector.tensor_scalar(
                                gT[h][:, sl], gT[h][:, sl], 0.5, 0.5,
                                _Alu.mult, _Alu.add,
                            )
                    else:
                        nc.vector.tensor_copy(dest[:, sl], pp[:])
                        nc.vector.tensor_copy(dsw[0:C, sl], pp[C : 2 * C, :])
                        nc.vector.tensor_copy(dsw[C : 2 * C, sl], pp[0:C, :])

            for tt in range(NKT):
                pv = projp.tile([P, P], F32, tag="pv")
                for ci in range(NCI):
                    nc.tensor.matmul(
                        pv[:],
                        xt_sb[ci][:, tt * P : (tt + 1) * P],
                        wv_sb[:, ci, :],
                        start=(ci == 0),
                        stop=(ci == NCI - 1),
                    )
                mcol = mc_sb[:, tt : tt + 1]
                v = vm[tt]
                nc.vector.tensor_scalar_mul(v[:, 0:C], pv[:, 0:C], mcol)
                nc.vector.tensor_scalar_mul(v[:, C + 1 : 2 * C + 1], pv[:, C : 2 * C], mcol)
                nc.vector.tensor_copy(v[:, C : C + 1], mcol)
                nc.vector.tensor_copy(v[:, 2 * C + 1 : 2 * C + 2], mcol)

        # ---------------- attention per head ----------------
        with (
            tc.tile_pool(name="spsum", bufs=3, space="PSUM") as spsum,
            tc.tile_pool(name="opsum", bufs=1, space="PSUM") as opsum,
            tc.tile_pool(name="ebp", bufs=8) as ebp,
            tc.tile_pool(name="pexp", bufs=4) as pexp,
            tc.tile_pool(name="ptp", bufs=8) as ptp,
            tc.tile_pool(name="epi", bufs=2) as epi,
        ):
            # The pair bias is injected into PSUM with an identity matmul
            # (DMA cannot write PSUM) and the qk matmul accumulates on top;
            # exp then reads the finished logits straight out of PSUM. This
            # keeps the elementwise bias work on the PE (which would
            # otherwise sit half-idle behind ACT) instead of a 48us DVE
            # multiply, and the higher PE duty keeps the HAM clock-gate from
            # re-throttling the array to 1.2 GHz mid-phase.
            for h in range(2):
                hpA = slice(C * h, C * (h + 1))          # rows of qT/kT
                hpB = slice(C * (1 - h), C * (2 - h))    # rows of qTs/kTs
                for j in range(NQH):
                    jsl = slice(QH * j, QH * (j + 1))
                    op_ = opsum.tile([C + 1, QH], F32, tag="op")
                    for kp in range(NKT // 2):
                        k0, k1 = 2 * kp, 2 * kp + 1
                        ks0 = slice(k0 * P, (k0 + 1) * P)
                        ks1 = slice(k1 * P, (k1 + 1) * P)
                        ebA = ebp.tile([P, QH], BF16, tag="eb")
                        nc.sync.dma_start(ebA[:], eb_t[h, ks0, jsl])
                        ebB = ebp.tile([P, QH], BF16, tag="eb")
                        nc.sync.dma_start(ebB[:], eb_t[h, ks1, jsl])
                        spA = spsum.tile([P, QH], F32, tag="sp")
                        spB = spsum.tile([P, QH], F32, tag="sp")
                        # Even k-tile (A): bias injected on the PE via an
                        # identity matmul, exp reads finished logits from
                        # PSUM. Odd k-tile (B): plain qk matmul; the host
                        # pre-exponentiated that bias slice, applied as a
                        # DVE multiply after exp. Splitting the bias work
                        # between PE and DVE keeps both at ~the ACT exp
                        # rate, and the PE duty stays high enough that the
                        # HAM clock gate holds 2.4 GHz through the phase.
                        for ch in range(QH // 512):
                            csl = slice(ch * 512, (ch + 1) * 512)
                            nc.tensor.matmul(
                                spA[:, csl], ident[:], ebA[:, csl],
                                start=True, stop=False,
                            )
                        for ch in range(QH // 512):
                            csl = slice(ch * 512, (ch + 1) * 512)
                            qs = QH * j + 512 * ch
                            nc.tensor.matmul(
                                spA[:, csl], kT[hpA, ks0],
                                qT[hpA, qs : qs + 512],
                                start=False, stop=True,
                            )
                            nc.tensor.matmul(
                                spB[:, csl], kTs[hpB, ks1],
                                qTs[hpB, qs : qs + 512],
                                start=True, stop=True,
                            )
                        ptA = ptp.tile([P, QH], BF16, tag="pt")
                        nc.scalar.activation(ptA[:], spA[:], Exp)
                        exB = pexp.tile([P, QH], BF16, tag="ex")
                        nc.scalar.activation(exB[:], spB[:], Exp)
                        ptB = ptp.tile([P, QH], BF16, tag="pt")
                        nc.vector.tensor_mul(ptB[:], exB[:], ebB[:])
                        for kt, pt in ((k0, ptA), (k1, ptB)):
                            for ch in range(QH // 512):
                                qs = 512 * ch
                                nc.tensor.matmul(
                                    op_[:, qs : qs + 512],
                                    vm[kt][:, (C + 1) * h : (C + 1) * (h + 1)],
                                    pt[:, qs : qs + 512],
                                    start=(kt == 0),
                                    stop=(kt == NKT - 1),
                                )

                    # epilogue for this q-half: rowsum -> per-q-tile columns
                    # (DRAM round trip; a single-row reciprocal would run on
                    # one DVE lane = ~13us), reciprocal on 128 lanes, gate
                    # applied unnormalized. The 1/rowsum scale is applied
                    # after the output projection (per-partition scalar
                    # there, since q sits on partitions).
                    rsum = epi.tile([1, QH], F32, tag="rsum")
                    nc.vector.tensor_copy(rsum[:], op_[C : C + 1, :])
                    dscr = drp.tile([1, QH], F32, tag="dscr")
                    nc.sync.dma_start(dscr[:], rsum[:])
                    nc.sync.dma_start(
                        rrec[h][:, (QH // P) * j : (QH // P) * (j + 1)],
                        dscr[0, :].rearrange("(t p) -> p t", p=P),
                    )
                    nc.vector.tensor_mul(
                        goun2[C * h : C * (h + 1), jsl], op_[0:C, :], gT[h][:, jsl]
                    )
            nc.vector.reciprocal(rrec[0][:], rrec[0][:])
            nc.vector.reciprocal(rrec[1][:], rrec[1][:])

        # ---------------- output projection ----------------
        with (
            tc.tile_pool(name="outp", bufs=2, space="PSUM") as outp,
            tc.tile_pool(name="outs", bufs=3) as outs,
        ):
            from concourse.alu_op_type import AluOpType

            for qt in range(NKT):
                qsl = slice(qt * P, (qt + 1) * P)
                po0 = outp.tile([P, C_IN], F32, tag="po0")
                nc.tensor.matmul(po0[:], goun2[0:C, qsl], wo_st[0:C, :],
                                 start=True, stop=True)
                po1 = outp.tile([P, C_IN], F32, tag="po1")
                nc.tensor.matmul(po1[:], goun2[C:P, qsl], wo_st[C:P, :],
                                 start=True, stop=True)
                t1 = outs.tile([P, C_IN], F32, tag="t1")
                nc.vector.tensor_scalar_mul(t1[:], po0[:], rrec[0][:, qt : qt + 1])
                ob = outs.tile([P, C_IN], F32, tag="ob")
                nc.vector.scalar_tensor_tensor(
                    ob[:], po1[:], rrec[1][:, qt : qt + 1], t1[:],
                    AluOpType.mult, AluOpType.add,
                )
                nc.sync.dma_start(out_t[qsl, :], ob[:])

    return nc


_NC_CACHE = None


def _get_nc():
    global _NC_CACHE
    if _NC_CACHE is None:
        _NC_CACHE = _build_nc()
    return _NC_CACHE


def _prepare_core(c, x, bias, attention_mask, Wq, Wk, Wv, Wg, bg, Wo):
    b = c // 4
    h1 = 2 * (c % 4)
    h2 = h1 + 1
    sl1 = slice(h1 * C, (h1 + 1) * C)
    sl2 = slice(h2 * C, (h2 + 1) * C)

    xt = np.ascontiguousarray(x[b].T).reshape(NCI, P, S).astype(NBF)

    def wsel(W, scale=1.0):
        w = np.concatenate([W[:, sl1], W[:, sl2]], axis=1)
        if scale != 1.0:
            w = w * scale
        return np.ascontiguousarray(w.reshape(NCI, P, P)).astype(NBF)

    wq = wsel(Wq, 1.0 / np.sqrt(C))
    wk = wsel(Wk)
    wg = wsel(Wg)
    wv = wsel(Wv)
    bgv = (0.5 * np.concatenate([bg[sl1], bg[sl2]])).reshape(P, 1).astype(np.float32)
    maskcol = np.ascontiguousarray(
        attention_mask[b].astype(np.float32).reshape(NKT, P).T
    )
    # transposed pair bias: ebias[j, k, q] = bias[b, h_j, q, k]; odd
    # k-tiles are pre-exponentiated (device applies them as exp(qk)*exp(b))
    eb = np.empty((2, S, S), dtype=NBF)
    for j, hh in enumerate((h1, h2)):
        bt = np.ascontiguousarray(bias[b, hh].T)
        bt4 = bt.reshape(NKT, P, S)
        for kt in range(NKT):
            if kt % 2:
                eb[j, kt * P : (kt + 1) * P] = np.exp(bt4[kt]).astype(NBF)
            else:
                eb[j, kt * P : (kt + 1) * P] = bt4[kt].astype(NBF)
    wo = np.concatenate([Wo[sl1, :], Wo[sl2, :]], 0).astype(NBF)

    return {
        "xt": xt,
        "wq": wq,
        "wk": wk,
        "wg": wg,
        "wv": wv,
        "bgv": bgv,
        "maskcol": maskcol,
        "ebias": eb,
        "wo": wo,
    }


def _run(inputs, trace=False, **kw):
    x = np.asarray(inputs["x"], dtype=np.float32)
    bias = np.asarray(inputs["bias"], dtype=np.float32)
    attention_mask = np.asarray(inputs["attention_mask"])
    Wq = np.asarray(inputs["Wq"], dtype=np.float32)
    Wk = np.asarray(inputs["Wk"], dtype=np.float32)
    Wv = np.asarray(inputs["Wv"], dtype=np.float32)
    Wg = np.asarray(inputs["Wg"], dtype=np.float32)
    bg = np.asarray(inputs["bg"], dtype=np.float32)
    Wo = np.asarray(inputs["Wo"], dtype=np.float32)
    bo = np.asarray(inputs["bo"], dtype=np.float32)

    in_maps = [None] * 8

    def prep(c):
        in_maps[c] = _prepare_core(c, x, bias, attention_mask, Wq, Wk, Wv, Wg, bg, Wo)

    threads = [threading.Thread(target=prep, args=(c,)) for c in range(8)]
    for t in threads:
        t.start()
    for t in threads:
        t.join()

    nc = _get_nc()
    res = run_bass_kernel_spmd(nc, in_maps, core_ids=list(range(8)), trace=trace, **kw)

    out = np.empty((B, S, C_IN), dtype=np.float32)
    for b in range(B):
        acc = res.results[4 * b]["out"].astype(np.float32)
        for c in range(4 * b + 1, 4 * b + 4):
            acc = acc + res.results[c]["out"]
        out[b] = acc + bo[None, :]
    return out, res


def kernel(**inputs) -> np.ndarray:
    return _run(inputs)[0]
